# revision 8
# baseline (speedup 1.0000x reference)
"""CopyGenerator kernel for Trainium2 (Bass/Tile), vocab-parallel over 8 cores.

Math (per core c, vocab shard [c*4000, (c+1)*4000)):
  gen_score = htgt @ emb_shard.T                       (PE)
  e = exp(gen_score)   [no max-sub; scores are O(3)]   (ACT, fused row-sum)
  Z = allreduce_add(sum_v e)                           (4KB collective)
  attention (all batches, duplicated on every core):
    qT/kT/k via q_w.T, s = qT.T@kT, attn = softmax(s/sqrt(d)),
    xT = k.T@attnT, c = x@w2 + b2 with w2 = f_w.T@copy_w.T (scores folded)
    a = sigmoid(c)
  copy_p shard = attn @ onehot(src_local)              (PE, exact)
  out = log(a*copy_p + (1-a)*e/Z)
      = Ln( c1 * (c2*copy_p + e) ),  c1=(1-a)/Z, c2=a*Z/(1-a)
"""

import sys

sys.path.insert(0, "/opt/trn_rl_repo")

import numpy as np

from concourse import bass, bacc, mybir
import concourse.tile as tile
from concourse.bass_utils import run_bass_kernel_spmd
from concourse.masks import make_identity

NT, NS, BS, D, V = 128, 128, 8, 512, 32000
NCORES = 8
VS = V // NCORES  # 4000 vocab per core
NCH = 8
CH = VS // NCH  # 500 cols per chunk (one PSUM bank)
P = 128
KC = D // P  # 4 contraction chunks
F32 = mybir.dt.float32
I32 = mybir.dt.int32
AF = mybir.ActivationFunctionType
ALU = mybir.AluOpType
INV_SQRT_D = 1.0 / float(np.sqrt(np.float32(D)))


def build_kernel():
    nc = bacc.Bacc(
        "TRN2",
        target_bir_lowering=False,
        debug=False,
        enable_asserts=False,
        num_devices=NCORES,
    )
    htgt = nc.dram_tensor("htgt", [NT, BS, D], F32, kind="ExternalInput").ap()
    hsrc = nc.dram_tensor("hsrc", [NS, BS, D], F32, kind="ExternalInput").ap()
    src = nc.dram_tensor("src_local", [NS, BS], I32, kind="ExternalInput").ap()
    emb = nc.dram_tensor("emb", [VS, D], F32, kind="ExternalInput").ap()
    q_w = nc.dram_tensor("q_w", [D, D], F32, kind="ExternalInput").ap()
    q_b = nc.dram_tensor("q_b", [D], F32, kind="ExternalInput").ap()
    f_w = nc.dram_tensor("f_w", [D, D], F32, kind="ExternalInput").ap()
    f_b = nc.dram_tensor("f_b", [D], F32, kind="ExternalInput").ap()
    copy_w = nc.dram_tensor("copy_w", [1, D], F32, kind="ExternalInput").ap()
    copy_b = nc.dram_tensor("copy_b", [1], F32, kind="ExternalInput").ap()
    out = nc.dram_tensor("out", [NT, BS, VS], F32, kind="ExternalOutput").ap()

    with tile.TileContext(nc) as tc:
        _emit(nc, tc, htgt, hsrc, src, emb, q_w, q_b, f_w, f_b, copy_w, copy_b, out)
    nc.compile()
    return nc


def _emit(nc, tc, htgt, hsrc, src, emb, q_w, q_b, f_w, f_b, copy_w, copy_b, out):
    with (
        tc.tile_pool(name="persist", bufs=1) as pw,
        tc.tile_pool(name="small", bufs=1) as psm,
        tc.tile_pool(name="ps_attn", bufs=2, space="PSUM") as ps_at,
        tc.tile_pool(name="ps_gen", bufs=3, space="PSUM") as ps_gen,
        tc.tile_pool(name="ps_cp", bufs=2, space="PSUM") as ps_cp,
        tc.tile_pool(name="dram", bufs=1, space="DRAM") as pdram,
    ):
        # ---- persistent SBUF ----
        htgtT = pw.tile([P, KC, BS, P], F32)  # (d, kc, b, t)
        attn_all = pw.tile([P, BS, NS], F32)  # (t, b, s) normalized attn
        attnT_all = pw.tile([P, BS, NT], F32)  # (s, b, t)
        a_all = pw.tile([P, BS], F32)
        src_sb = pw.tile([P, BS], I32)
        w2_sb = pw.tile([P, KC], F32)
        b2_sb = pw.tile([1, 1], F32)
        identity = pw.tile([P, P], F32)
        ones_row = pw.tile([1, P], F32)
        zparts = pw.tile([P, BS, NCH], F32)
        zloc = pw.tile([P, BS], F32)
        zg_sb = pw.tile([P, BS], F32)
        c1_sb = pw.tile([P, BS], F32)
        c2_sb = pw.tile([P, BS], F32)

        zin = pdram.tile([P, BS], F32)
        zout = pdram.tile([P, BS], F32)

        make_identity(nc, identity[:])
        nc.vector.memset(ones_row[:], 1.0)

        htgtT_r = htgt.rearrange("t b (kc d) -> d kc b t", d=P)
        for kc in range(KC):
            for b in range(BS):
                nc.sync.dma_start(out=htgtT[:, kc, b], in_=htgtT_r[:, kc, b])
        nc.sync.dma_start(out=src_sb[:], in_=src)

        with tc.tile_pool(name="attn_w", bufs=1) as paw, tc.tile_pool(
            name="attn_t", bufs=2
        ) as pat:
            hsrcT = paw.tile([P, KC, BS, P], F32)  # (d, kc, b, s)
            qwT = paw.tile([P, KC, D], F32)  # (d, kc, i)
            fw = paw.tile([P, KC, D], F32)  # (j, jc, i)
            cwT = paw.tile([P, KC], F32)  # (j, jc)
            fbT = paw.tile([P, KC], F32)  # (j, jc)
            qb_row = paw.tile([1, D], F32)
            cb_sb = paw.tile([1, 1], F32)

            hsrcT_r = hsrc.rearrange("s b (kc d) -> d kc b s", d=P)
            for kc in range(KC):
                for b in range(BS):
                    nc.sync.dma_start(out=hsrcT[:, kc, b], in_=hsrcT_r[:, kc, b])
            qwT_r = q_w.rearrange("i (kc d) -> d kc i", d=P)
            fw_r = f_w.rearrange("(jc j) i -> j jc i", j=P)
            for kc in range(KC):
                nc.sync.dma_start(out=qwT[:, kc], in_=qwT_r[:, kc])
                nc.sync.dma_start(out=fw[:, kc], in_=fw_r[:, kc])
            nc.sync.dma_start(
                out=cwT[:], in_=copy_w.rearrange("o (jc j) -> j (jc o)", j=P)
            )
            nc.sync.dma_start(out=fbT[:], in_=f_b.rearrange("(jc j) -> j jc", j=P))
            nc.sync.dma_start(out=qb_row[:], in_=q_b.unsqueeze(0))
            nc.sync.dma_start(out=cb_sb[:], in_=copy_b.unsqueeze(0))

            # ---- fold f_w/copy_w: w2 = f_w.T @ copy_w.T ; b2 = copy_w@f_b + copy_b
            for ic in range(KC):
                w2_ps = ps_at.tile([P, 1], F32, tag="at")
                for jc in range(KC):
                    nc.tensor.matmul(
                        out=w2_ps[:],
                        lhsT=fw[:, jc, ic * P : (ic + 1) * P],
                        rhs=cwT[:, jc : jc + 1],
                        start=(jc == 0),
                        stop=(jc == KC - 1),
                    )
                nc.vector.tensor_copy(out=w2_sb[:, ic : ic + 1], in_=w2_ps[:])
            b2_ps = ps_at.tile([1, 1], F32, tag="at")
            for jc in range(KC):
                nc.tensor.matmul(
                    out=b2_ps[:],
                    lhsT=cwT[:, jc : jc + 1],
                    rhs=fbT[:, jc : jc + 1],
                    start=(jc == 0),
                    stop=(jc == KC - 1),
                )
            nc.vector.tensor_add(out=b2_sb[:], in0=b2_ps[:], in1=cb_sb[:])

            # ---- attention per batch (duplicated on all cores) ----
            for b in range(BS):
                qT_sb = pat.tile([P, KC, P], F32, tag="qT")  # (i, ic, t)
                kT_sb = pat.tile([P, KC, P], F32, tag="kT")  # (i, ic, s)
                k_sb = pat.tile([P, D], F32, tag="k")  # (s, i)
                xT_sb = pat.tile([P, KC, P], F32, tag="xT")  # (i, ic, t)

                for ic in range(KC):
                    qT_ps = ps_at.tile([P, P], F32, tag="at")
                    for kc in range(KC):
                        nc.tensor.matmul(
                            out=qT_ps[:],
                            lhsT=qwT[:, kc, ic * P : (ic + 1) * P],
                            rhs=htgtT[:, kc, b, :],
                            start=(kc == 0),
                            stop=False,
                        )
                    nc.tensor.matmul(
                        out=qT_ps[:],
                        lhsT=qb_row[:, ic * P : (ic + 1) * P],
                        rhs=ones_row[:],
                        start=False,
                        stop=True,
                    )
                    nc.vector.tensor_copy(out=qT_sb[:, ic, :], in_=qT_ps[:])

                    kT_ps = ps_at.tile([P, P], F32, tag="at")
                    for kc in range(KC):
                        nc.tensor.matmul(
                            out=kT_ps[:],
                            lhsT=qwT[:, kc, ic * P : (ic + 1) * P],
                            rhs=hsrcT[:, kc, b, :],
                            start=(kc == 0),
                            stop=False,
                        )
                    nc.tensor.matmul(
                        out=kT_ps[:],
                        lhsT=qb_row[:, ic * P : (ic + 1) * P],
                        rhs=ones_row[:],
                        start=False,
                        stop=True,
                    )
                    nc.vector.tensor_copy(out=kT_sb[:, ic, :], in_=kT_ps[:])

                k_ps = ps_at.tile([P, D], F32, tag="at")
                for kc in range(KC):
                    nc.tensor.matmul(
                        out=k_ps[:],
                        lhsT=hsrcT[:, kc, b, :],
                        rhs=qwT[:, kc, :],
                        start=(kc == 0),
                        stop=False,
                    )
                nc.tensor.matmul(
                    out=k_ps[:], lhsT=ones_row[:], rhs=qb_row[:], start=False, stop=True
                )
                nc.vector.tensor_copy(out=k_sb[:], in_=k_ps[:])

                s_ps = ps_at.tile([P, P], F32, tag="at")
                for ic in range(KC):
                    nc.tensor.matmul(
                        out=s_ps[:],
                        lhsT=qT_sb[:, ic, :],
                        rhs=kT_sb[:, ic, :],
                        start=(ic == 0),
                        stop=(ic == KC - 1),
                    )
                m_col = psm.tile([P, 1], F32, tag="m")
                negm = psm.tile([P, 1], F32, tag="negm")
                zatt = psm.tile([P, 1], F32, tag="zatt")
                rz = psm.tile([P, 1], F32, tag="rz")
                nc.vector.reduce_max(
                    out=m_col[:], in_=s_ps[:], axis=mybir.AxisListType.X
                )
                nc.vector.tensor_scalar_mul(negm[:], m_col[:], -INV_SQRT_D)
                nc.scalar.activation(
                    out=attn_all[:, b, :],
                    in_=s_ps[:],
                    func=AF.Exp,
                    bias=negm[:],
                    scale=INV_SQRT_D,
                    accum_out=zatt[:],
                )
                nc.vector.reciprocal(rz[:], zatt[:])
                nc.vector.tensor_scalar_mul(attn_all[:, b, :], attn_all[:, b, :], rz[:])

                t_ps = ps_at.tile([P, P], F32, tag="at")
                nc.tensor.transpose(t_ps[:], attn_all[:, b, :], identity[:])
                nc.vector.tensor_copy(out=attnT_all[:, b, :], in_=t_ps[:])

                for ic in range(KC):
                    x_ps = ps_at.tile([P, P], F32, tag="at")
                    nc.tensor.matmul(
                        out=x_ps[:],
                        lhsT=k_sb[:, ic * P : (ic + 1) * P],
                        rhs=attnT_all[:, b, :],
                        start=True,
                        stop=True,
                    )
                    nc.vector.tensor_copy(out=xT_sb[:, ic, :], in_=x_ps[:])

                c_ps = ps_at.tile([P, 1], F32, tag="at")
                for ic in range(KC):
                    nc.tensor.matmul(
                        out=c_ps[:],
                        lhsT=xT_sb[:, ic, :],
                        rhs=w2_sb[:, ic : ic + 1],
                        start=(ic == 0),
                        stop=False,
                    )
                nc.tensor.matmul(
                    out=c_ps[:], lhsT=ones_row[:], rhs=b2_sb[:], start=False, stop=True
                )
                nc.scalar.activation(
                    out=a_all[:, b : b + 1], in_=c_ps[:], func=AF.Sigmoid
                )

        # ---- pass 1: gen scores -> e, row sums ----
        with (
            tc.tile_pool(name="e", bufs=1) as pe,
            tc.tile_pool(name="embT", bufs=2) as pemb,
            tc.tile_pool(name="io", bufs=2) as pio,
        ):
            e_all = pe.tile([P, BS, VS], F32)
            embT_r = emb.rearrange("v (kc d) -> d kc v", d=P)
            for n in range(NCH):
                embT_n = pemb.tile([P, KC, CH], F32, tag="embT")
                for kc in range(KC):
                    nc.sync.dma_start(
                        out=embT_n[:, kc], in_=embT_r[:, kc, n * CH : (n + 1) * CH]
                    )
                for b in range(BS):
                    g_ps = ps_gen.tile([P, CH], F32, tag="g")
                    for kc in range(KC):
                        nc.tensor.matmul(
                            out=g_ps[:],
                            lhsT=htgtT[:, kc, b, :],
                            rhs=embT_n[:, kc, :],
                            start=(kc == 0),
                            stop=(kc == KC - 1),
                        )
                    nc.scalar.activation(
                        out=e_all[:, b, n * CH : (n + 1) * CH],
                        in_=g_ps[:],
                        func=AF.Exp,
                        accum_out=zparts[:, b, n : n + 1],
                    )

            # ---- global Z via AllReduce ----
            nc.vector.reduce_sum(out=zloc[:], in_=zparts[:], axis=mybir.AxisListType.X)
            nc.sync.dma_start(out=zin[:], in_=zloc[:])
            nc.gpsimd.collective_compute(
                "AllReduce",
                ALU.add,
                replica_groups=[list(range(NCORES))],
                ins=[zin[:].opt()],
                outs=[zout[:].opt()],
            )
            nc.sync.dma_start(out=zg_sb[:], in_=zout[:])

            # c1 = (1-a)/Z ; c2 = a*Z/(1-a)
            one_minus_a = psm.tile([P, BS], F32, tag="oma")
            r_oma = psm.tile([P, BS], F32, tag="roma")
            r_z = psm.tile([P, BS], F32, tag="rzg")
            az = psm.tile([P, BS], F32, tag="az")
            nc.vector.tensor_scalar(
                out=one_minus_a[:],
                in0=a_all[:],
                scalar1=-1.0,
                scalar2=1.0,
                op0=ALU.mult,
                op1=ALU.add,
            )
            nc.vector.reciprocal(r_oma[:], one_minus_a[:])
            nc.vector.reciprocal(r_z[:], zg_sb[:])
            nc.vector.tensor_tensor(
                out=c1_sb[:], in0=one_minus_a[:], in1=r_z[:], op=ALU.mult
            )
            nc.vector.tensor_tensor(out=az[:], in0=a_all[:], in1=zg_sb[:], op=ALU.mult)
            nc.vector.tensor_tensor(out=c2_sb[:], in0=az[:], in1=r_oma[:], op=ALU.mult)

            # ---- pass 2: copy_p + blend + log + store ----
            for n in range(NCH):
                iota_n = pio.tile([P, CH], I32, tag="iota")
                nc.gpsimd.iota(
                    iota_n[:], pattern=[[1, CH]], base=n * CH, channel_multiplier=0
                )
                for b in range(BS):
                    onehot = pio.tile([P, CH], F32, tag="oh")
                    nc.vector.tensor_tensor(
                        out=onehot[:],
                        in0=src_sb[:, b : b + 1].to_broadcast([P, CH]),
                        in1=iota_n[:],
                        op=ALU.is_equal,
                    )
                    cp_ps = ps_cp.tile([P, CH], F32, tag="cp")
                    nc.tensor.matmul(
                        out=cp_ps[:],
                        lhsT=attnT_all[:, b, :],
                        rhs=onehot[:],
                        start=True,
                        stop=True,
                    )
                    blend = pio.tile([P, CH], F32, tag="blend")
                    nc.vector.scalar_tensor_tensor(
                        out=blend[:],
                        in0=cp_ps[:],
                        scalar=c2_sb[:, b : b + 1],
                        in1=e_all[:, b, n * CH : (n + 1) * CH],
                        op0=ALU.mult,
                        op1=ALU.add,
                    )
                    outt = pio.tile([P, CH], F32, tag="outt")
                    nc.scalar.activation(
                        out=outt[:], in_=blend[:], func=AF.Ln, scale=c1_sb[:, b : b + 1]
                    )
                    nc.sync.dma_start(
                        out=out[:, b, n * CH : (n + 1) * CH], in_=outt[:]
                    )


_NC_CACHE = []


def _get_nc():
    if not _NC_CACHE:
        _NC_CACHE.append(build_kernel())
    return _NC_CACHE[0]


def _make_in_maps(inputs):
    htgt = np.ascontiguousarray(np.asarray(inputs["htgt"], dtype=np.float32))
    hsrc = np.ascontiguousarray(np.asarray(inputs["hsrc"], dtype=np.float32))
    src = np.ascontiguousarray(np.asarray(inputs["src"]).astype(np.int32))
    emb = np.ascontiguousarray(np.asarray(inputs["emb_weight"], dtype=np.float32))
    q_w = np.ascontiguousarray(np.asarray(inputs["q_w"], dtype=np.float32))
    q_b = np.ascontiguousarray(np.asarray(inputs["q_b"], dtype=np.float32))
    f_w = np.ascontiguousarray(np.asarray(inputs["f_w"], dtype=np.float32))
    f_b = np.ascontiguousarray(np.asarray(inputs["f_b"], dtype=np.float32))
    copy_w = np.ascontiguousarray(np.asarray(inputs["copy_w"], dtype=np.float32))
    copy_b = np.ascontiguousarray(np.asarray(inputs["copy_b"], dtype=np.float32))

    in_maps = []
    for c in range(NCORES):
        in_maps.append(
            {
                "htgt": htgt,
                "hsrc": hsrc,
                "src_local": (src - np.int32(c * VS)).astype(np.int32),
                "emb": np.ascontiguousarray(emb[c * VS : (c + 1) * VS]),
                "q_w": q_w,
                "q_b": q_b,
                "f_w": f_w,
                "f_b": f_b,
                "copy_w": copy_w,
                "copy_b": copy_b,
            }
        )
    return in_maps


def kernel(**inputs):
    in_maps = _make_in_maps(inputs)
    nc = _get_nc()
    res = run_bass_kernel_spmd(nc, in_maps, list(range(NCORES))).results
    return np.concatenate([res[c]["out"] for c in range(NCORES)], axis=2)
